# revision 7
# baseline (speedup 1.0000x reference)
"""Causal attention with padding mask on 8 Trainium2 NeuronCores.

Problem: B=8, S=2048, D=512, fp32, single head.
  scores = (Q @ K^T) / sqrt(D), causal + per-key padding mask, softmax,
  out = P @ V.

Sharding: pure data-parallel over batch -- each of the 8 cores computes one
batch element; no collectives.

Per-core algorithm ("ST layout" flash attention, no max-subtraction):
  Scores are computed TRANSPOSED (keys on partitions, queries on the free
  dim):  ST[j, i] = sum_d K[j,d] Q[i,d] = matmul(lhsT=K^T chunk, rhs=Q^T).
  This makes exp(ST) directly usable as the stationary operand of the PV
  matmul (out[i,:] += sum_j P^T[j,i] V[j,:]) -- no per-tile P transposes.
  The padding mask folds into the exp as a per-partition bias
  (exp(scale*s + bias_j), bias_j = -30000 for masked keys -> exp == 0), the
  causal mask is a single precomputed triangular multiplicative tile applied
  to diagonal chunks, and the softmax denominator is a ones-column matmul
  sharing the PV stationary.  Since scores/sqrt(D) are O(5), exp() cannot
  overflow fp32 and the usual max-subtraction pass is skipped entirely.

  Q^T / K^T are produced on-chip with PE transposes (DMA transpose does not
  support 4-byte dtypes); all matmuls run as float32r (full-rate fp32).
"""

import sys

sys.path.insert(0, "/opt/trn_rl_repo")

import numpy as np

S = 2048
D = 512
NCORES = 8
SCALE = 1.0 / float(np.sqrt(float(D)))
NEG = -30000.0

SC = S // 128  # 16 key-chunks / q-subtiles of 128
DC = D // 128  # 4 d-chunks of 128
G = S // 512   # 4 q-blocks of 512


def _build():
    import concourse.tile as tile
    from concourse import bacc, mybir
    from contextlib import ExitStack

    f32 = mybir.dt.float32
    f32r = mybir.dt.float32r
    i32 = mybir.dt.int32
    Exp = mybir.ActivationFunctionType.Exp

    nc = bacc.Bacc("TRN2", target_bir_lowering=False, debug=False,
                   num_devices=NCORES)
    q_d = nc.dram_tensor("query", [S, D], f32r, kind="ExternalInput").ap()
    k_d = nc.dram_tensor("key", [S, D], f32r, kind="ExternalInput").ap()
    v_d = nc.dram_tensor("value", [S, D], f32r, kind="ExternalInput").ap()
    m_d = nc.dram_tensor("attention_mask", [S], i32, kind="ExternalInput").ap()
    o_d = nc.dram_tensor("out", [S, D], f32, kind="ExternalOutput").ap()

    with ExitStack() as ctx:
        tc = ctx.enter_context(tile.TileContext(nc))
        persist = ctx.enter_context(tc.tile_pool(name="persist", bufs=1))
        natp = ctx.enter_context(tc.tile_pool(name="nat", bufs=16))
        ptp = ctx.enter_context(tc.tile_pool(name="pt", bufs=3))
        outp = ctx.enter_context(tc.tile_pool(name="ostage", bufs=3))
        smallp = ctx.enter_context(tc.tile_pool(name="small", bufs=2))
        pst = ctx.enter_context(tc.tile_pool(name="pst", bufs=3, space="PSUM"))
        pout = ctx.enter_context(tc.tile_pool(name="pout", bufs=1, space="PSUM"))
        pden = ctx.enter_context(tc.tile_pool(name="pden", bufs=1, space="PSUM"))

        QT = [persist.tile([128, S], f32r, tag=f"qt{d}", name=f"qt{d}")
              for d in range(DC)]
        KT = [persist.tile([128, S], f32r, tag=f"kt{d}", name=f"kt{d}")
              for d in range(DC)]
        V = [persist.tile([128, D], f32r, tag=f"v{c}", name=f"v{c}")
             for c in range(SC)]
        ident = persist.tile([128, 128], f32r, tag="ident", name="ident")
        tri = persist.tile([128, 128], f32r, tag="tri", name="tri")
        ones = persist.tile([128, 2], f32r, tag="ones", name="ones")
        identf = persist.tile([128, 128], f32, tag="identf", name="identf")
        trif = persist.tile([128, 128], f32, tag="trif", name="trif")
        onesf = persist.tile([128, 2], f32, tag="onesf", name="onesf")
        biasc = persist.tile([128, SC], f32, tag="biasc", name="biasc")
        maskf = persist.tile([128, SC], f32, tag="maskf", name="maskf")
        maski = persist.tile([128, SC], i32, tag="maski", name="maski")

        # --- constants (gpsimd can only write f32; DVE copies round to f32r) ---
        nc.gpsimd.memset(identf[:], 0.0)
        nc.gpsimd.affine_select(
            out=identf[:], in_=identf[:], compare_op=mybir.AluOpType.not_equal,
            fill=1.0, base=0, pattern=[[-1, 128]], channel_multiplier=1)
        # tri[j, i] = 1.0 where j <= i else 0.0  (causal keep, ST layout)
        nc.gpsimd.memset(trif[:], 1.0)
        nc.gpsimd.affine_select(
            out=trif[:], in_=trif[:], compare_op=mybir.AluOpType.is_ge,
            fill=0.0, base=0, pattern=[[1, 128]], channel_multiplier=-1)
        nc.gpsimd.memset(onesf[:], 1.0)
        nc.vector.tensor_copy(ident[:], identf[:])
        nc.vector.tensor_copy(tri[:], trif[:])
        nc.vector.tensor_copy(ones[:], onesf[:])

        # padding-mask exp bias: biasc[p, c] = (mask[128c+p] - 1) * (-NEG)
        nc.sync.dma_start(out=maski[:], in_=m_d.rearrange("(c p) -> p c", p=128))
        nc.vector.tensor_copy(maskf[:], maski[:])
        nc.vector.tensor_scalar(
            out=biasc[:], in0=maskf[:], scalar1=-NEG, scalar2=NEG,
            op0=mybir.AluOpType.mult, op1=mybir.AluOpType.add)

        # --- input DMAs (natural layout; K/Q staged for PE transposes) ---
        Kn = [None] * SC
        Qn = [None] * SC
        for g in range(G):
            for t in range(4 * g, 4 * g + 4):
                Kn[t] = natp.tile([128, D], f32r, tag="nat", name=f"kn{t}")
                nc.sync.dma_start(out=Kn[t][:], in_=k_d[t * 128:(t + 1) * 128, :])
            for t in range(4 * g, 4 * g + 4):
                Qn[t] = natp.tile([128, D], f32r, tag="nat", name=f"qn{t}")
                nc.sync.dma_start(out=Qn[t][:], in_=q_d[t * 128:(t + 1) * 128, :])
            for t in range(4 * g, 4 * g + 4):
                nc.sync.dma_start(out=V[t][:], in_=v_d[t * 128:(t + 1) * 128, :])

        def emit_transposes(g, src, dst):
            # transpose s-chunks 4g..4g+3 into dst[dc][:, 512g:512g+512]
            # one PSUM bank holds 4 transposed 128x128 chunks; a zero region
            # admits a single accumulation group, so only the first transpose
            # starts it and only the last stops it (writes are disjoint).
            for dc in range(DC):
                ps = pst.tile([128, 512], f32, tag="st", name=f"tp{g}{dc}")
                for t in range(4):
                    st = 4 * g + t
                    nc.tensor.matmul(
                        out=ps[:, t * 128:(t + 1) * 128].bitcast(f32r),
                        lhsT=src[st][:, dc * 128:(dc + 1) * 128],
                        rhs=ident[:],
                        is_transpose=True,
                        start=(t == 0), stop=(t == 3))
                nc.vector.tensor_copy(
                    dst[dc][:, 512 * g:512 * (g + 1)], ps[:].bitcast(f32r))

        # --- main loop over q-blocks of 512 ---
        for g in range(G):
            emit_transposes(g, Kn, KT)
            emit_transposes(g, Qn, QT)
            nchunks = 4 * g + 4
            ST_t = [None] * nchunks
            PT_t = [None] * nchunks
            qoffs = [0] * nchunks
            OUTPS = [pout.tile([128, D], f32, tag=f"o{i}", name=f"o{g}{i}")
                     for i in range(4)]
            DEN = pden.tile([128, 8], f32, tag="den", name=f"den{g}")

            def emit_qk(c, g=g, nchunks=nchunks, ST_t=ST_t, PT_t=PT_t,
                        qoffs=qoffs):
                r = c - 4 * g
                # trim fully-masked leading q columns on diagonal chunks when
                # the remaining width stays >= 256 (float32r full-rate limit)
                qoff = 128 * r if r in (1, 2) else 0
                qoffs[c] = qoff
                n = 512 - qoff
                stt = pst.tile([128, 512], f32, tag="st", name=f"st{g}_{c}")
                ST_t[c] = stt
                for dc in range(DC):
                    nc.tensor.matmul(
                        out=stt[:, 0:n],
                        lhsT=KT[dc][:, c * 128:(c + 1) * 128],
                        rhs=QT[dc][:, 512 * g + qoff:512 * (g + 1)],
                        start=(dc == 0), stop=(dc == DC - 1))
                ptt = ptp.tile([128, 512], f32r, tag="pt", name=f"pt{g}_{c}")
                PT_t[c] = ptt
                nc.scalar.activation(
                    out=ptt[:, 0:n], in_=stt[:, 0:n], func=Exp,
                    bias=biasc[:, c:c + 1], scale=SCALE)
                if r >= 0:
                    loc = 128 * r - qoff
                    nc.vector.tensor_mul(
                        ptt[:, loc:loc + 128], ptt[:, loc:loc + 128], tri[:])

            def emit_pv(c, g=g, ST_t=ST_t, PT_t=PT_t, qoffs=qoffs,
                        OUTPS=OUTPS, DEN=DEN):
                qoff = qoffs[c]
                s_first = max(c, 4 * g)
                for s in range(s_first, 4 * g + 4):
                    i = s - 4 * g
                    sloc = 128 * i - qoff
                    nc.tensor.matmul(
                        out=OUTPS[i][:],
                        lhsT=PT_t[c][:, sloc:sloc + 128],
                        rhs=V[c][:],
                        start=(c == 0), stop=(c == s))
                    # all 4 DEN columns share one PSUM zero region: single
                    # group started by (c==0, i==0), stopped by the last den
                    # matmul of the block (c==4g+3 emits only s==4g+3).
                    nc.tensor.matmul(
                        out=DEN[:, 2 * i:2 * i + 2],
                        lhsT=PT_t[c][:, sloc:sloc + 128],
                        rhs=ones[:],
                        start=(c == 0 and i == 0),
                        stop=(c == 4 * g + 3 and s == 4 * g + 3))

            emit_qk(0)
            for c in range(1, nchunks):
                emit_qk(c)
                emit_pv(c - 1)
            emit_pv(nchunks - 1)

            recip = smallp.tile([128, 8], f32, tag="recip", name=f"recip{g}")
            nc.vector.reciprocal(recip[:], DEN[:])
            for i in range(4):
                ost = outp.tile([128, D], f32, tag="ost", name=f"ost{g}{i}")
                nc.vector.tensor_scalar_mul(ost[:], OUTPS[i][:], recip[:, 2 * i:2 * i + 1])
                s = 4 * g + i
                nc.sync.dma_start(out=o_d[s * 128:(s + 1) * 128, :], in_=ost[:])

    nc.compile()
    return nc


_NC_CACHE = []


def _get_nc():
    if not _NC_CACHE:
        _NC_CACHE.append(_build())
    return _NC_CACHE[0]


def run(inputs, trace=False):
    from concourse import bass_utils

    nc = _get_nc()
    in_maps = []
    for i in range(NCORES):
        in_maps.append({
            "query": np.ascontiguousarray(inputs["query"][i], dtype=np.float32),
            "key": np.ascontiguousarray(inputs["key"][i], dtype=np.float32),
            "value": np.ascontiguousarray(inputs["value"][i], dtype=np.float32),
            "attention_mask": np.ascontiguousarray(
                inputs["attention_mask"][i], dtype=np.int32),
        })
    res = bass_utils.run_bass_kernel_spmd(
        nc, in_maps, core_ids=list(range(NCORES)), trace=trace)
    out = np.stack([np.asarray(res.results[i]["out"]) for i in range(NCORES)])
    return out.astype(np.float32), res


def kernel(query, key, value, attention_mask):
    out, _ = run({"query": query, "key": key, "value": value,
                  "attention_mask": attention_mask})
    return out


# revision 8
# speedup vs baseline: 657.0165x; 657.0165x over previous
"""Causal attention with padding mask on 8 Trainium2 NeuronCores.

Problem: B=8, S=2048, D=512, fp32, single head.
  scores = (Q @ K^T) / sqrt(D), causal + per-key padding mask, softmax,
  out = P @ V.

Sharding: pure data-parallel over batch -- each of the 8 cores computes one
batch element; no collectives.

Per-core algorithm ("ST layout" flash attention, no max-subtraction):
  Scores are computed TRANSPOSED (keys on partitions, queries on the free
  dim):  ST[j, i] = sum_d K[j,d] Q[i,d] = matmul(lhsT=K^T chunk, rhs=Q^T).
  This makes exp(ST) directly usable as the stationary operand of the PV
  matmul (out[i,:] += sum_j P^T[j,i] V[j,:]) -- no per-tile P transposes.
  The padding mask folds into the exp as a per-partition bias
  (exp(scale*s + bias_j), bias_j = -30000 for masked keys -> exp == 0), the
  causal mask is a single precomputed triangular multiplicative tile applied
  to diagonal chunks, and the softmax denominator is a ones-column matmul
  sharing the PV stationary.  Since scores/sqrt(D) are O(5), exp() cannot
  overflow fp32 and the usual max-subtraction pass is skipped entirely.

  Q^T / K^T are produced on-chip with PE transposes (DMA transpose does not
  support 4-byte dtypes); all matmuls run as float32r (full-rate fp32).
"""

import sys

sys.path.insert(0, "/opt/trn_rl_repo")

import numpy as np

S = 2048
D = 512
NCORES = 8
SCALE = 1.0 / float(np.sqrt(float(D)))
NEG = -30000.0

SC = S // 128  # 16 key-chunks / q-subtiles of 128
DC = D // 128  # 4 d-chunks of 128
G = S // 512   # 4 q-blocks of 512


def _build(reps=1):
    import concourse.tile as tile
    from concourse import bacc, mybir
    from contextlib import ExitStack

    f32 = mybir.dt.float32
    f32r = mybir.dt.float32r
    i32 = mybir.dt.int32
    Exp = mybir.ActivationFunctionType.Exp

    nc = bacc.Bacc("TRN2", target_bir_lowering=False, debug=False,
                   num_devices=NCORES)
    q_d = nc.dram_tensor("query", [S, D], f32r, kind="ExternalInput").ap()
    k_d = nc.dram_tensor("key", [S, D], f32r, kind="ExternalInput").ap()
    v_d = nc.dram_tensor("value", [S, D], f32r, kind="ExternalInput").ap()
    m_d = nc.dram_tensor("attention_mask", [S], i32, kind="ExternalInput").ap()
    o_d = nc.dram_tensor("out", [S, D], f32, kind="ExternalOutput").ap()

    with ExitStack() as ctx:
        tc = ctx.enter_context(tile.TileContext(nc))
        if reps > 1:
            ctx.enter_context(tc.For_i(0, reps, 1))
        persist = ctx.enter_context(tc.tile_pool(name="persist", bufs=1))
        natp = ctx.enter_context(tc.tile_pool(name="nat", bufs=16))
        ptp = ctx.enter_context(tc.tile_pool(name="pt", bufs=3))
        outp = ctx.enter_context(tc.tile_pool(name="ostage", bufs=3))
        smallp = ctx.enter_context(tc.tile_pool(name="small", bufs=2))
        pst = ctx.enter_context(tc.tile_pool(name="pst", bufs=3, space="PSUM"))
        pout = ctx.enter_context(tc.tile_pool(name="pout", bufs=1, space="PSUM"))
        pden = ctx.enter_context(tc.tile_pool(name="pden", bufs=1, space="PSUM"))

        QT = [persist.tile([128, S], f32r, tag=f"qt{d}", name=f"qt{d}")
              for d in range(DC)]
        KT = [persist.tile([128, S], f32r, tag=f"kt{d}", name=f"kt{d}")
              for d in range(DC)]
        V = [persist.tile([128, D], f32r, tag=f"v{c}", name=f"v{c}")
             for c in range(SC)]
        ident = persist.tile([128, 128], f32r, tag="ident", name="ident")
        tri = persist.tile([128, 128], f32r, tag="tri", name="tri")
        ones = persist.tile([128, 2], f32r, tag="ones", name="ones")
        identf = persist.tile([128, 128], f32, tag="identf", name="identf")
        trif = persist.tile([128, 128], f32, tag="trif", name="trif")
        onesf = persist.tile([128, 2], f32, tag="onesf", name="onesf")
        biasc = persist.tile([128, SC], f32, tag="biasc", name="biasc")
        maskf = persist.tile([128, SC], f32, tag="maskf", name="maskf")
        maski = persist.tile([128, SC], i32, tag="maski", name="maski")

        # --- constants (gpsimd can only write f32; DVE copies round to f32r) ---
        nc.gpsimd.memset(identf[:], 0.0)
        nc.gpsimd.affine_select(
            out=identf[:], in_=identf[:], compare_op=mybir.AluOpType.not_equal,
            fill=1.0, base=0, pattern=[[-1, 128]], channel_multiplier=1)
        # tri[j, i] = 1.0 where j <= i else 0.0  (causal keep, ST layout)
        nc.gpsimd.memset(trif[:], 1.0)
        nc.gpsimd.affine_select(
            out=trif[:], in_=trif[:], compare_op=mybir.AluOpType.is_ge,
            fill=0.0, base=0, pattern=[[1, 128]], channel_multiplier=-1)
        nc.gpsimd.memset(onesf[:], 1.0)
        nc.vector.tensor_copy(ident[:], identf[:])
        nc.vector.tensor_copy(tri[:], trif[:])
        nc.vector.tensor_copy(ones[:], onesf[:])

        # padding-mask exp bias: biasc[p, c] = (mask[128c+p] - 1) * (-NEG)
        nc.sync.dma_start(out=maski[:], in_=m_d.rearrange("(c p) -> p c", p=128))
        nc.vector.tensor_copy(maskf[:], maski[:])
        nc.vector.tensor_scalar(
            out=biasc[:], in0=maskf[:], scalar1=-NEG, scalar2=NEG,
            op0=mybir.AluOpType.mult, op1=mybir.AluOpType.add)

        # --- input DMAs (natural layout; K/Q staged for PE transposes) ---
        Kn = [None] * SC
        Qn = [None] * SC
        for g in range(G):
            for t in range(4 * g, 4 * g + 4):
                Kn[t] = natp.tile([128, D], f32r, tag="nat", name=f"kn{t}")
                nc.sync.dma_start(out=Kn[t][:], in_=k_d[t * 128:(t + 1) * 128, :])
            for t in range(4 * g, 4 * g + 4):
                Qn[t] = natp.tile([128, D], f32r, tag="nat", name=f"qn{t}")
                nc.sync.dma_start(out=Qn[t][:], in_=q_d[t * 128:(t + 1) * 128, :])
            for t in range(4 * g, 4 * g + 4):
                nc.sync.dma_start(out=V[t][:], in_=v_d[t * 128:(t + 1) * 128, :])

        def emit_transposes(g, src, dst):
            # transpose s-chunks 4g..4g+3 into dst[dc][:, 512g:512g+512]
            # one PSUM bank holds 4 transposed 128x128 chunks; a zero region
            # admits a single accumulation group, so only the first transpose
            # starts it and only the last stops it (writes are disjoint).
            for dc in range(DC):
                ps = pst.tile([128, 512], f32, tag="st", name=f"tp{g}{dc}")
                for t in range(4):
                    st = 4 * g + t
                    nc.tensor.matmul(
                        out=ps[:, t * 128:(t + 1) * 128].bitcast(f32r),
                        lhsT=src[st][:, dc * 128:(dc + 1) * 128],
                        rhs=ident[:],
                        is_transpose=True,
                        start=(t == 0), stop=(t == 3))
                nc.vector.tensor_copy(
                    dst[dc][:, 512 * g:512 * (g + 1)], ps[:].bitcast(f32r))

        # --- main loop over q-blocks of 512 ---
        for g in range(G):
            emit_transposes(g, Kn, KT)
            emit_transposes(g, Qn, QT)
            nchunks = 4 * g + 4
            ST_t = [None] * nchunks
            PT_t = [None] * nchunks
            qoffs = [0] * nchunks
            OUTPS = [pout.tile([128, D], f32, tag=f"o{i}", name=f"o{g}{i}")
                     for i in range(4)]
            DEN = pden.tile([128, 8], f32, tag="den", name=f"den{g}")

            def emit_qk(c, g=g, nchunks=nchunks, ST_t=ST_t, PT_t=PT_t,
                        qoffs=qoffs):
                r = c - 4 * g
                # trim fully-masked leading q columns on diagonal chunks when
                # the remaining width stays >= 256 (float32r full-rate limit)
                qoff = 128 * r if r in (1, 2) else 0
                qoffs[c] = qoff
                n = 512 - qoff
                stt = pst.tile([128, 512], f32, tag="st", name=f"st{g}_{c}")
                ST_t[c] = stt
                for dc in range(DC):
                    nc.tensor.matmul(
                        out=stt[:, 0:n],
                        lhsT=KT[dc][:, c * 128:(c + 1) * 128],
                        rhs=QT[dc][:, 512 * g + qoff:512 * (g + 1)],
                        start=(dc == 0), stop=(dc == DC - 1))
                ptt = ptp.tile([128, 512], f32r, tag="pt", name=f"pt{g}_{c}")
                PT_t[c] = ptt
                nc.scalar.activation(
                    out=ptt[:, 0:n], in_=stt[:, 0:n], func=Exp,
                    bias=biasc[:, c:c + 1], scale=SCALE)
                if r >= 0:
                    loc = 128 * r - qoff
                    nc.vector.tensor_mul(
                        ptt[:, loc:loc + 128], ptt[:, loc:loc + 128], tri[:])

            def emit_pv(c, g=g, ST_t=ST_t, PT_t=PT_t, qoffs=qoffs,
                        OUTPS=OUTPS, DEN=DEN):
                qoff = qoffs[c]
                s_first = max(c, 4 * g)
                for s in range(s_first, 4 * g + 4):
                    i = s - 4 * g
                    sloc = 128 * i - qoff
                    nc.tensor.matmul(
                        out=OUTPS[i][:],
                        lhsT=PT_t[c][:, sloc:sloc + 128],
                        rhs=V[c][:],
                        start=(c == 0), stop=(c == s))
                    # all 4 DEN columns share one PSUM zero region: single
                    # group started by (c==0, i==0), stopped by the last den
                    # matmul of the block (c==4g+3 emits only s==4g+3).
                    nc.tensor.matmul(
                        out=DEN[:, 2 * i:2 * i + 2],
                        lhsT=PT_t[c][:, sloc:sloc + 128],
                        rhs=ones[:],
                        start=(c == 0 and i == 0),
                        stop=(c == 4 * g + 3 and s == 4 * g + 3))

            emit_qk(0)
            for c in range(1, nchunks):
                emit_qk(c)
                emit_pv(c - 1)
            emit_pv(nchunks - 1)

            recip = smallp.tile([128, 8], f32, tag="recip", name=f"recip{g}")
            nc.vector.reciprocal(recip[:], DEN[:])
            for i in range(4):
                ost = outp.tile([128, D], f32, tag="ost", name=f"ost{g}{i}")
                nc.vector.tensor_scalar_mul(ost[:], OUTPS[i][:], recip[:, 2 * i:2 * i + 1])
                s = 4 * g + i
                nc.sync.dma_start(out=o_d[s * 128:(s + 1) * 128, :], in_=ost[:])

    nc.compile()
    return nc


_NC_CACHE = {}


def _get_nc(reps=1):
    if reps not in _NC_CACHE:
        _NC_CACHE[reps] = _build(reps)
    return _NC_CACHE[reps]


def run(inputs, trace=False):
    from concourse import bass_utils

    nc = _get_nc()
    in_maps = []
    for i in range(NCORES):
        in_maps.append({
            "query": np.ascontiguousarray(inputs["query"][i], dtype=np.float32),
            "key": np.ascontiguousarray(inputs["key"][i], dtype=np.float32),
            "value": np.ascontiguousarray(inputs["value"][i], dtype=np.float32),
            "attention_mask": np.ascontiguousarray(
                inputs["attention_mask"][i], dtype=np.int32),
        })
    res = bass_utils.run_bass_kernel_spmd(
        nc, in_maps, core_ids=list(range(NCORES)), trace=trace)
    out = np.stack([np.asarray(res.results[i]["out"]) for i in range(NCORES)])
    return out.astype(np.float32), res


def kernel(query, key, value, attention_mask):
    out, _ = run({"query": query, "key": key, "value": value,
                  "attention_mask": attention_mask})
    return out


# revision 9
# speedup vs baseline: 670.0888x; 1.0199x over previous
"""Causal attention with padding mask on 8 Trainium2 NeuronCores.

Problem: B=8, S=2048, D=512, fp32, single head.
  scores = (Q @ K^T) / sqrt(D), causal + per-key padding mask, softmax,
  out = P @ V.

Sharding: pure data-parallel over batch -- each of the 8 cores computes one
batch element; no collectives.

Per-core algorithm ("ST layout" flash attention, no max-subtraction):
  Scores are computed TRANSPOSED (keys on partitions, queries on the free
  dim):  ST[j, i] = sum_d K[j,d] Q[i,d] = matmul(lhsT=K^T chunk, rhs=Q^T).
  This makes exp(ST) directly usable as the stationary operand of the PV
  matmul (out[i,:] += sum_j P^T[j,i] V[j,:]) -- no per-tile P transposes.
  The padding mask folds into the exp as a per-partition bias
  (exp(scale*s + bias_j), bias_j = -30000 for masked keys -> exp == 0), the
  causal mask is a single precomputed triangular multiplicative tile applied
  to diagonal chunks, and the softmax denominator is a ones-column matmul
  sharing the PV stationary.  Since scores/sqrt(D) are O(5), exp() cannot
  overflow fp32 and the usual max-subtraction pass is skipped entirely.

  Q^T / K^T are produced on-chip with PE transposes (DMA transpose does not
  support 4-byte dtypes); all matmuls run as float32r (full-rate fp32).
"""

import sys

sys.path.insert(0, "/opt/trn_rl_repo")

import numpy as np

S = 2048
D = 512
NCORES = 8
SCALE = 1.0 / float(np.sqrt(float(D)))
NEG = -30000.0

SC = S // 128  # 16 key-chunks / q-subtiles of 128
DC = D // 128  # 4 d-chunks of 128
G = S // 512   # 4 q-blocks of 512


def _build(reps=1):
    import concourse.tile as tile
    from concourse import bacc, mybir
    from contextlib import ExitStack

    f32 = mybir.dt.float32
    f32r = mybir.dt.float32r
    i32 = mybir.dt.int32
    Exp = mybir.ActivationFunctionType.Exp

    nc = bacc.Bacc("TRN2", target_bir_lowering=False, debug=False,
                   num_devices=NCORES)
    q_d = nc.dram_tensor("query", [S, D], f32r, kind="ExternalInput").ap()
    k_d = nc.dram_tensor("key", [S, D], f32r, kind="ExternalInput").ap()
    v_d = nc.dram_tensor("value", [S, D], f32r, kind="ExternalInput").ap()
    m_d = nc.dram_tensor("attention_mask", [S], i32, kind="ExternalInput").ap()
    o_d = nc.dram_tensor("out", [S, D], f32, kind="ExternalOutput").ap()

    with ExitStack() as ctx:
        tc = ctx.enter_context(tile.TileContext(nc))
        if reps > 1:
            ctx.enter_context(tc.For_i(0, reps, 1))
        persist = ctx.enter_context(tc.tile_pool(name="persist", bufs=1))
        natp = ctx.enter_context(tc.tile_pool(name="nat", bufs=6))
        ptp = ctx.enter_context(tc.tile_pool(name="pt", bufs=3))
        outp = ctx.enter_context(tc.tile_pool(name="ostage", bufs=2))
        smallp = ctx.enter_context(tc.tile_pool(name="small", bufs=2))
        pst = ctx.enter_context(tc.tile_pool(name="pst", bufs=3, space="PSUM"))
        pout = ctx.enter_context(tc.tile_pool(name="pout", bufs=1, space="PSUM"))
        pden = ctx.enter_context(tc.tile_pool(name="pden", bufs=1, space="PSUM"))

        QT = [persist.tile([128, S], f32r, tag=f"qt{d}", name=f"qt{d}")
              for d in range(DC)]
        KT = [persist.tile([128, S], f32r, tag=f"kt{d}", name=f"kt{d}")
              for d in range(DC)]
        VG = [persist.tile([128, 4, D], f32r, tag=f"vg{gg}", name=f"vg{gg}")
              for gg in range(G)]
        ident = persist.tile([128, 128], f32r, tag="ident", name="ident")
        tri = persist.tile([128, 128], f32r, tag="tri", name="tri")
        ones = persist.tile([128, 2], f32r, tag="ones", name="ones")
        identf = persist.tile([128, 128], f32, tag="identf", name="identf")
        trif = persist.tile([128, 128], f32, tag="trif", name="trif")
        onesf = persist.tile([128, 2], f32, tag="onesf", name="onesf")
        biasc = persist.tile([128, SC], f32, tag="biasc", name="biasc")
        maskf = persist.tile([128, SC], f32, tag="maskf", name="maskf")
        maski = persist.tile([128, SC], i32, tag="maski", name="maski")

        # --- constants (gpsimd can only write f32; DVE copies round to f32r) ---
        nc.gpsimd.memset(identf[:], 0.0)
        nc.gpsimd.affine_select(
            out=identf[:], in_=identf[:], compare_op=mybir.AluOpType.not_equal,
            fill=1.0, base=0, pattern=[[-1, 128]], channel_multiplier=1)
        # tri[j, i] = 1.0 where j <= i else 0.0  (causal keep, ST layout)
        nc.gpsimd.memset(trif[:], 1.0)
        nc.gpsimd.affine_select(
            out=trif[:], in_=trif[:], compare_op=mybir.AluOpType.is_ge,
            fill=0.0, base=0, pattern=[[1, 128]], channel_multiplier=-1)
        nc.gpsimd.memset(onesf[:], 1.0)
        nc.vector.tensor_copy(ident[:], identf[:])
        nc.vector.tensor_copy(tri[:], trif[:])
        nc.vector.tensor_copy(ones[:], onesf[:])

        # padding-mask exp bias: biasc[p, c] = (mask[128c+p] - 1) * (-NEG)
        nc.sync.dma_start(out=maski[:], in_=m_d.rearrange("(c p) -> p c", p=128))
        nc.vector.tensor_copy(maskf[:], maski[:])
        nc.vector.tensor_scalar(
            out=biasc[:], in0=maskf[:], scalar1=-NEG, scalar2=NEG,
            op0=mybir.AluOpType.mult, op1=mybir.AluOpType.add)

        # --- input DMAs: 1MB group transfers (>=1MiB for ~78% of DMA peak),
        # K/Q on the SP HWDGE ring, V on the gpsimd SWDGE ring ---
        k_g = k_d.rearrange("(c p) d -> p c d", p=128)
        q_g = q_d.rearrange("(c p) d -> p c d", p=128)
        v_g = v_d.rearrange("(c p) d -> p c d", p=128)
        KnG = [None] * G
        QnG = [None] * G
        for g in range(G):
            KnG[g] = natp.tile([128, 4, D], f32r, tag="nat", name=f"kng{g}")
            nc.sync.dma_start(out=KnG[g][:], in_=k_g[:, 4 * g:4 * g + 4, :])
            QnG[g] = natp.tile([128, 4, D], f32r, tag="nat", name=f"qng{g}")
            nc.sync.dma_start(out=QnG[g][:], in_=q_g[:, 4 * g:4 * g + 4, :])
            nc.gpsimd.dma_start(out=VG[g][:], in_=v_g[:, 4 * g:4 * g + 4, :])

        def emit_transposes(g, src, dst):
            # transpose s-chunks 4g..4g+3 into dst[dc][:, 512g:512g+512]
            # one PSUM bank holds 4 transposed 128x128 chunks; a zero region
            # admits a single accumulation group, so only the first transpose
            # starts it and only the last stops it (writes are disjoint).
            for dc in range(DC):
                ps = pst.tile([128, 512], f32, tag="st", name=f"tp{g}{dc}")
                for t in range(4):
                    nc.tensor.matmul(
                        out=ps[:, t * 128:(t + 1) * 128].bitcast(f32r),
                        lhsT=src[g][:, t, dc * 128:(dc + 1) * 128],
                        rhs=ident[:],
                        is_transpose=True,
                        start=(t == 0), stop=(t == 3))
                nc.vector.tensor_copy(
                    dst[dc][:, 512 * g:512 * (g + 1)], ps[:].bitcast(f32r))

        # --- main loop over q-blocks of 512 ---
        for g in range(G):
            emit_transposes(g, KnG, KT)
            emit_transposes(g, QnG, QT)
            nchunks = 4 * g + 4
            ST_t = [None] * nchunks
            PT_t = [None] * nchunks
            qoffs = [0] * nchunks
            OUTPS = [pout.tile([128, D], f32, tag=f"o{i}", name=f"o{g}{i}")
                     for i in range(4)]
            DEN = pden.tile([128, 8], f32, tag="den", name=f"den{g}")

            def emit_qk(c, g=g, nchunks=nchunks, ST_t=ST_t, PT_t=PT_t,
                        qoffs=qoffs):
                r = c - 4 * g
                # trim fully-masked leading q columns on diagonal chunks when
                # the remaining width stays >= 256 (float32r full-rate limit)
                qoff = 128 * r if r in (1, 2) else 0
                qoffs[c] = qoff
                n = 512 - qoff
                stt = pst.tile([128, 512], f32, tag="st", name=f"st{g}_{c}")
                ST_t[c] = stt
                for dc in range(DC):
                    nc.tensor.matmul(
                        out=stt[:, 0:n],
                        lhsT=KT[dc][:, c * 128:(c + 1) * 128],
                        rhs=QT[dc][:, 512 * g + qoff:512 * (g + 1)],
                        start=(dc == 0), stop=(dc == DC - 1))
                ptt = ptp.tile([128, 512], f32r, tag="pt", name=f"pt{g}_{c}")
                PT_t[c] = ptt
                nc.scalar.activation(
                    out=ptt[:, 0:n], in_=stt[:, 0:n], func=Exp,
                    bias=biasc[:, c:c + 1], scale=SCALE)
                if r >= 0:
                    loc = 128 * r - qoff
                    nc.vector.tensor_mul(
                        ptt[:, loc:loc + 128], ptt[:, loc:loc + 128], tri[:])

            def emit_pv(c, g=g, ST_t=ST_t, PT_t=PT_t, qoffs=qoffs,
                        OUTPS=OUTPS, DEN=DEN):
                qoff = qoffs[c]
                s_first = max(c, 4 * g)
                for s in range(s_first, 4 * g + 4):
                    i = s - 4 * g
                    sloc = 128 * i - qoff
                    nc.tensor.matmul(
                        out=OUTPS[i][:],
                        lhsT=PT_t[c][:, sloc:sloc + 128],
                        rhs=VG[c // 4][:, c % 4, :],
                        start=(c == 0), stop=(c == s))
                    # all 4 DEN columns share one PSUM zero region: single
                    # group started by (c==0, i==0), stopped by the last den
                    # matmul of the block (c==4g+3 emits only s==4g+3).
                    nc.tensor.matmul(
                        out=DEN[:, 2 * i:2 * i + 2],
                        lhsT=PT_t[c][:, sloc:sloc + 128],
                        rhs=ones[:],
                        start=(c == 0 and i == 0),
                        stop=(c == 4 * g + 3 and s == 4 * g + 3))

            emit_qk(0)
            for c in range(1, nchunks):
                emit_qk(c)
                emit_pv(c - 1)
            emit_pv(nchunks - 1)

            recip = smallp.tile([128, 8], f32, tag="recip", name=f"recip{g}")
            nc.vector.reciprocal(recip[:], DEN[:])
            ost = outp.tile([128, 4, D], f32, tag="ost", name=f"ost{g}")
            for i in range(4):
                nc.vector.tensor_scalar_mul(
                    ost[:, i, :], OUTPS[i][:], recip[:, 2 * i:2 * i + 1])
            o_g = o_d.rearrange("(s p) d -> p s d", p=128)
            nc.scalar.dma_start(out=o_g[:, 4 * g:4 * g + 4, :], in_=ost[:])

    nc.compile()
    return nc


_NC_CACHE = {}


def _get_nc(reps=1):
    if reps not in _NC_CACHE:
        _NC_CACHE[reps] = _build(reps)
    return _NC_CACHE[reps]


def run(inputs, trace=False):
    from concourse import bass_utils

    nc = _get_nc()
    in_maps = []
    for i in range(NCORES):
        in_maps.append({
            "query": np.ascontiguousarray(inputs["query"][i], dtype=np.float32),
            "key": np.ascontiguousarray(inputs["key"][i], dtype=np.float32),
            "value": np.ascontiguousarray(inputs["value"][i], dtype=np.float32),
            "attention_mask": np.ascontiguousarray(
                inputs["attention_mask"][i], dtype=np.int32),
        })
    res = bass_utils.run_bass_kernel_spmd(
        nc, in_maps, core_ids=list(range(NCORES)), trace=trace)
    out = np.stack([np.asarray(res.results[i]["out"]) for i in range(NCORES)])
    return out.astype(np.float32), res


def kernel(query, key, value, attention_mask):
    out, _ = run({"query": query, "key": key, "value": value,
                  "attention_mask": attention_mask})
    return out


# revision 18
# speedup vs baseline: 838.0101x; 1.2506x over previous
"""Causal attention with padding mask on 8 Trainium2 NeuronCores.

Problem: B=8, S=2048, D=512, fp32, single head.
  scores = (Q @ K^T) / sqrt(D), causal + per-key padding mask, softmax,
  out = P @ V.

Sharding: pure data-parallel over batch -- each of the 8 cores computes one
batch element; no collectives.

Per-core algorithm ("ST layout" flash attention, no max-subtraction):
  Scores are computed TRANSPOSED (keys on partitions, queries on the free
  dim):  ST[j, i] = sum_d K[j,d] Q[i,d] = matmul(lhsT=K^T chunk, rhs=Q^T).
  This makes exp(ST) directly usable as the stationary operand of the PV
  matmul (out[i,:] += sum_j P^T[j,i] V[j,:]) -- no per-tile P transposes.
  The padding mask folds into the exp as a per-partition bias
  (exp(scale*s + bias_j), bias_j = -30000 for masked keys -> exp == 0), the
  causal mask is a single precomputed triangular multiplicative tile applied
  to diagonal chunks, and the softmax denominator is a ones-column matmul
  sharing the PV stationary.  Since scores/sqrt(D) are O(5), exp() cannot
  overflow fp32 and the usual max-subtraction pass is skipped entirely.

  Q^T / K^T are produced on-chip with PE transposes (DMA transpose does not
  support 4-byte dtypes).  All matmuls run in bf16 (measured ~2.3x faster
  than the float32r path on this toolchain; end-to-end rel err ~3e-3 vs the
  2e-2 gate): natural K/Q tiles are DMA'd as f32, cast to bf16 on DVE, and
  transposed at 1 cycle/row.  V is cast f32->bf16 during its SWDGE DMA.
  Probe/flag parameters on _build() are timing experiments; the production
  configuration is _build(reps=1, use_bf16=True, bf16_nat=True).
"""

import sys

sys.path.insert(0, "/opt/trn_rl_repo")

import numpy as np

S = 2048
D = 512
NCORES = 8
SCALE = 1.0 / float(np.sqrt(float(D)))
NEG = -30000.0

SC = S // 128  # 16 key-chunks / q-subtiles of 128
DC = D // 128  # 4 d-chunks of 128
G = S // 512   # 4 q-blocks of 512


def _build(reps=1, use_bf16=True, bf16_nat=False, probe=None, spread=False):
    import concourse.tile as tile
    from concourse import bacc, mybir
    from contextlib import ExitStack

    f32 = mybir.dt.float32
    f32r = mybir.dt.bfloat16 if use_bf16 else mybir.dt.float32r
    i32 = mybir.dt.int32
    tdt = mybir.dt.float32 if use_bf16 else mybir.dt.float32r
    Exp = mybir.ActivationFunctionType.Exp

    nc = bacc.Bacc("TRN2", target_bir_lowering=False, debug=False,
                   num_devices=NCORES)
    q_d = nc.dram_tensor("query", [S, D], tdt, kind="ExternalInput").ap()
    k_d = nc.dram_tensor("key", [S, D], tdt, kind="ExternalInput").ap()
    v_d = nc.dram_tensor("value", [S, D], tdt, kind="ExternalInput").ap()
    m_d = nc.dram_tensor("attention_mask", [S], i32, kind="ExternalInput").ap()
    o_d = nc.dram_tensor("out", [S, D], f32, kind="ExternalOutput").ap()

    with ExitStack() as ctx:
        tc = ctx.enter_context(tile.TileContext(nc))
        if reps > 1:
            ctx.enter_context(tc.For_i(0, reps, 1))
        persist = ctx.enter_context(tc.tile_pool(name="persist", bufs=1))
        natp = ctx.enter_context(tc.tile_pool(name="nat", bufs=6))
        ptp = ctx.enter_context(tc.tile_pool(name="pt", bufs=3))
        outp = ctx.enter_context(tc.tile_pool(name="ostage", bufs=2))
        smallp = ctx.enter_context(tc.tile_pool(name="small", bufs=2))
        pst = ctx.enter_context(tc.tile_pool(name="pst", bufs=3, space="PSUM"))
        pout = ctx.enter_context(tc.tile_pool(name="pout", bufs=1, space="PSUM"))
        pden = ctx.enter_context(tc.tile_pool(name="pden", bufs=1, space="PSUM"))

        QT = [persist.tile([128, S], f32r, tag=f"qt{d}", name=f"qt{d}")
              for d in range(DC)]
        KT = [persist.tile([128, S], f32r, tag=f"kt{d}", name=f"kt{d}")
              for d in range(DC)]
        VG = [persist.tile([128, 4, D], f32r, tag=f"vg{gg}", name=f"vg{gg}")
              for gg in range(G)]
        ident = persist.tile([128, 128], f32, tag="ident", name="ident")
        identb = persist.tile([128, 128], f32r, tag="identb", name="identb")
        tri = persist.tile([128, 128], f32r, tag="tri", name="tri")
        ones = persist.tile([128, 2], f32r, tag="ones", name="ones")
        identf = persist.tile([128, 128], f32, tag="identf", name="identf")
        trif = persist.tile([128, 128], f32, tag="trif", name="trif")
        onesf = persist.tile([128, 2], f32, tag="onesf", name="onesf")
        biasc = persist.tile([128, SC], f32, tag="biasc", name="biasc")
        maskf = persist.tile([128, SC], f32, tag="maskf", name="maskf")
        maski = persist.tile([128, SC], i32, tag="maski", name="maski")

        # --- constants (gpsimd can only write f32; DVE copies round to f32r) ---
        nc.gpsimd.memset(identf[:], 0.0)
        nc.gpsimd.affine_select(
            out=identf[:], in_=identf[:], compare_op=mybir.AluOpType.not_equal,
            fill=1.0, base=0, pattern=[[-1, 128]], channel_multiplier=1)
        # tri[j, i] = 1.0 where j <= i else 0.0  (causal keep, ST layout)
        nc.gpsimd.memset(trif[:], 1.0)
        nc.gpsimd.affine_select(
            out=trif[:], in_=trif[:], compare_op=mybir.AluOpType.is_ge,
            fill=0.0, base=0, pattern=[[1, 128]], channel_multiplier=-1)
        nc.gpsimd.memset(onesf[:], 1.0)
        nc.vector.tensor_copy(ident[:], identf[:])
        nc.vector.tensor_copy(identb[:], identf[:])
        nc.vector.tensor_copy(tri[:], trif[:])
        nc.vector.tensor_copy(ones[:], onesf[:])

        # padding-mask exp bias: biasc[p, c] = (mask[128c+p] - 1) * (-NEG)
        nc.sync.dma_start(out=maski[:], in_=m_d.rearrange("(c p) -> p c", p=128))
        nc.vector.tensor_copy(maskf[:], maski[:])
        nc.vector.tensor_scalar(
            out=biasc[:], in0=maskf[:], scalar1=-NEG, scalar2=NEG,
            op0=mybir.AluOpType.mult, op1=mybir.AluOpType.add)

        # --- input DMAs: 1MB group transfers (>=1MiB for ~78% of DMA peak),
        # K/Q on the SP HWDGE ring, V on the gpsimd SWDGE ring ---
        k_g = k_d.rearrange("(c p) d -> p c d", p=128)
        q_g = q_d.rearrange("(c p) d -> p c d", p=128)
        v_g = v_d.rearrange("(c p) d -> p c d", p=128)
        KnG = [None] * G
        QnG = [None] * G
        for g in range(G):
            KnG[g] = natp.tile([128, 4, D], tdt, tag="nat", name=f"kng{g}")
            nc.sync.dma_start(out=KnG[g][:], in_=k_g[:, 4 * g:4 * g + 4, :])
            QnG[g] = natp.tile([128, 4, D], tdt, tag="nat", name=f"qng{g}")
            (nc.scalar if spread else nc.sync).dma_start(
                out=QnG[g][:], in_=q_g[:, 4 * g:4 * g + 4, :])
            nc.gpsimd.dma_start(out=VG[g][:], in_=v_g[:, 4 * g:4 * g + 4, :])

        natb = ctx.enter_context(tc.tile_pool(
            name="natb", bufs=8 if probe == "notrans" else 4))
        ptc = persist.tile([128, 512], f32r, tag="ptc", name="ptc")
        nc.vector.tensor_copy(ptc[:, 0:128], tri[:])
        nc.vector.tensor_copy(ptc[:, 128:256], tri[:])
        nc.vector.tensor_copy(ptc[:, 256:384], tri[:])
        nc.vector.tensor_copy(ptc[:, 384:512], tri[:])
        KnB = [None] * G
        QnB = [None] * G

        def emit_convert(g):
            KnB[g] = natb.tile([128, 4, D], f32r, tag="natb", name=f"knb{g}")
            nc.vector.tensor_copy(KnB[g][:], KnG[g][:])
            QnB[g] = natb.tile([128, 4, D], f32r, tag="natb", name=f"qnb{g}")
            nc.vector.tensor_copy(QnB[g][:], QnG[g][:])

        def emit_transposes(g, src, dst):
            # transpose s-chunks 4g..4g+3 into dst[dc][:, 512g:512g+512]
            # one PSUM bank holds 4 transposed 128x128 chunks; a zero region
            # admits a single accumulation group, so only the first transpose
            # starts it and only the last stops it (writes are disjoint).
            tp_dt = f32r if bf16_nat else tdt
            for dc in range(DC):
                ps = pst.tile([128, 512], tp_dt, tag="st", name=f"tp{g}{dc}")
                for t in range(4):
                    nc.tensor.matmul(
                        out=ps[:, t * 128:(t + 1) * 128],
                        lhsT=src[g][:, t, dc * 128:(dc + 1) * 128],
                        rhs=identb[:] if bf16_nat else ident[:],
                        is_transpose=True,
                        start=(t == 0), stop=(t == 3))
                nc.vector.tensor_copy(
                    dst[dc][:, 512 * g:512 * (g + 1)], ps[:])

        # --- main loop over q-blocks of 512 ---
        if probe == "dmaonly":
            for g in range(G):
                ost = outp.tile([128, 4, D], f32, tag="ost", name=f"ost{g}")
                nc.vector.tensor_copy(ost[:, 0, :], KnG[g][:, 0, :])
                nc.vector.tensor_copy(ost[:, 1, :], QnG[g][:, 1, :])
                nc.vector.tensor_copy(ost[:, 2, :], VG[g][:, 2, :])
                nc.vector.tensor_copy(ost[:, 3, :], KnG[g][:, 3, :])
                o_g2 = o_d.rearrange("(s p) d -> p s d", p=128)
                nc.scalar.dma_start(out=o_g2[:, 4 * g:4 * g + 4, :], in_=ost[:])
            _finish = True
        else:
            _finish = False
        if bf16_nat and not _finish:
            emit_convert(0)
            if probe == "notrans":
                for gg in range(1, G):
                    emit_convert(gg)
        for g in range(G if not _finish else 0):
            if probe != "notrans":
                srcK = KnB if bf16_nat else KnG
                emit_transposes(g, srcK, KT)
                srcQ = QnB if bf16_nat else QnG
                emit_transposes(g, srcQ, QT)
            nchunks = 4 * g + 4
            ST_t = [None] * nchunks
            PT_t = [None] * nchunks
            qoffs = [0] * nchunks
            OUTPS = [pout.tile([128, D], f32, tag=f"o{i}", name=f"o{g}{i}")
                     for i in range(4)]
            DEN = pden.tile([128, 8], f32, tag="den", name=f"den{g}")

            def emit_qk(c, g=g, nchunks=nchunks, ST_t=ST_t, PT_t=PT_t,
                        qoffs=qoffs):
                r = c - 4 * g
                # trim fully-masked leading q columns on diagonal chunks when
                # the remaining width stays >= 256 (float32r full-rate limit)
                qoff = 128 * r if r in (1, 2, 3) else 0
                qoffs[c] = qoff
                n = 512 - qoff
                stt = pst.tile([128, 512], f32, tag="st", name=f"st{g}_{c}")
                ST_t[c] = stt
                if probe != "pvonly":
                    for dc in range(DC):
                        if probe == "notrans":
                            lhsT = KnB[c // 4][:, c % 4, dc * 128:(dc + 1) * 128]
                            rhs = QnB[g][:, 0, 0:512 - qoff]
                        else:
                            lhsT = KT[dc][:, c * 128:(c + 1) * 128]
                            rhs = QT[dc][:, 512 * g + qoff:512 * (g + 1)]
                        nc.tensor.matmul(
                            out=stt[:, 0:n], lhsT=lhsT, rhs=rhs,
                            start=(dc == 0), stop=(dc == DC - 1))
                if probe == "noexp" or probe == "pvonly":
                    PT_t[c] = ptc
                    return
                ptt = ptp.tile([128, 512], f32r, tag="pt", name=f"pt{g}_{c}")
                PT_t[c] = ptt
                nc.scalar.activation(
                    out=ptt[:, 0:n], in_=stt[:, 0:n], func=Exp,
                    bias=biasc[:, c:c + 1], scale=SCALE)
                if r >= 0:
                    loc = 128 * r - qoff
                    nc.vector.tensor_mul(
                        ptt[:, loc:loc + 128], ptt[:, loc:loc + 128], tri[:])

            def emit_pv(c, g=g, ST_t=ST_t, PT_t=PT_t, qoffs=qoffs,
                        OUTPS=OUTPS, DEN=DEN):
                if probe == "qkonly":
                    return
                qoff = qoffs[c]
                s_first = max(c, 4 * g)
                for s in range(s_first, 4 * g + 4):
                    i = s - 4 * g
                    sloc = 128 * i - qoff
                    nc.tensor.matmul(
                        out=OUTPS[i][:],
                        lhsT=PT_t[c][:, sloc:sloc + 128],
                        rhs=VG[c // 4][:, c % 4, :],
                        start=(c == 0), stop=(c == s))
                    # all 4 DEN columns share one PSUM zero region: single
                    # group started by (c==0, i==0), stopped by the last den
                    # matmul of the block (c==4g+3 emits only s==4g+3).
                    nc.tensor.matmul(
                        out=DEN[:, 2 * i:2 * i + 2],
                        lhsT=PT_t[c][:, sloc:sloc + 128],
                        rhs=ones[:],
                        start=(c == 0 and i == 0),
                        stop=(c == 4 * g + 3 and s == 4 * g + 3))

            emit_qk(0)
            for c in range(1, nchunks):
                emit_qk(c)
                emit_pv(c - 1)
                if c == 1 and bf16_nat and probe != "notrans" and g + 1 < G:
                    emit_convert(g + 1)
            emit_pv(nchunks - 1)

            ost = outp.tile([128, 4, D], f32, tag="ost", name=f"ost{g}")
            if probe == "qkonly":
                for i in range(4):
                    nc.vector.tensor_copy(ost[:, i, :], ptc[:])
            else:
                recip = smallp.tile([128, 8], f32, tag="recip", name=f"recip{g}")
                nc.vector.reciprocal(recip[:], DEN[:])
                for i in range(4):
                    nc.vector.tensor_scalar_mul(
                        ost[:, i, :], OUTPS[i][:], recip[:, 2 * i:2 * i + 1])
            o_g = o_d.rearrange("(s p) d -> p s d", p=128)
            nc.scalar.dma_start(out=o_g[:, 4 * g:4 * g + 4, :], in_=ost[:])

    nc.compile()
    return nc


_NC_CACHE = {}


def _get_nc(reps=1, use_bf16=True, bf16_nat=True, spread=False):
    key = (reps, use_bf16, bf16_nat, spread)
    if key not in _NC_CACHE:
        _NC_CACHE[key] = _build(reps, use_bf16, bf16_nat, spread=spread)
    return _NC_CACHE[key]


def run(inputs, trace=False):
    from concourse import bass_utils

    nc = _get_nc()
    in_maps = []
    for i in range(NCORES):
        in_maps.append({
            "query": np.ascontiguousarray(inputs["query"][i], dtype=np.float32),
            "key": np.ascontiguousarray(inputs["key"][i], dtype=np.float32),
            "value": np.ascontiguousarray(inputs["value"][i], dtype=np.float32),
            "attention_mask": np.ascontiguousarray(
                inputs["attention_mask"][i], dtype=np.int32),
        })
    res = bass_utils.run_bass_kernel_spmd(
        nc, in_maps, core_ids=list(range(NCORES)), trace=trace)
    out = np.stack([np.asarray(res.results[i]["out"]) for i in range(NCORES)])
    return out.astype(np.float32), res


def kernel(query, key, value, attention_mask):
    out, _ = run({"query": query, "key": key, "value": value,
                  "attention_mask": attention_mask})
    return out
